# revision 10
# baseline (speedup 1.0000x reference)
"""Trainium2 Bass kernel for nn_MembraneLayer: h = x @ w followed by a
double first-order recurrence over time, producing (syn_rec, mem_rec).

Sharding: data-parallel over batch. 8 cores x 64 batches each.

Per-core layout (hardcoded). Columns are T-MAJOR INTERLEAVED per quartet
of 16 batches: within quartet q, column 16*t + j holds (batch 16q+j,
time t). This makes every DVE op a rank<=3 access pattern (the BIR
verifier rejects rank-4), makes matmuls contiguous rank-2 (the t+1
shifted write is a uniform +16 column offset), and aligns PSUM banks
exactly (512 f32 = 16 batches x 32 time steps).

  xt16 [700, 6400] fp16   x in [C, (q, t, j)] order (host-prepped)
  w16  [700, 512]  fp16
  pows [128, 24]   f32    per (partition, d_tile): alpha, a^2, a^4, beta,
                          b^2, b^4 (6 slots per d_tile)
  syn  [512, 6400] fp16   out: syn_rec in [D, (q, t, j)] layout
  mem  [512, 6400] fp16   out: mem_rec[t+1]/(1-beta), written +16 shifted;
                          t=0 zeroed + (1-beta) scale applied on host

Per quartet, per d_tile (128 rows of D): single-pass fp16 matmuls
accumulate h into a 4-bank PSUM tile at column 16(t+1)+j. The time
recurrences use a K=4 Blelloch block-scan instead of a plain serial scan
(the DVE serial scan runs at ~2.2 ns/col; full-rate STT ops at ~0.52):
  down:  g1[i] = a*h[2i]   + h[2i+1]           (STT, per-partition scalar)
         g2[u] = a^2*g1[2u] + g1[2u+1]         (STT, written batch-major)
  scan:  s4 = serial scan of g2 with coef a^4  (1/4 of the columns)
  copy:  s4 -> syn16 at t=4u+3 (fp16)
  up:    syn[4u+1] = a^2*syn[4u-1] + g1[2u]    (STT)
         syn[2i]   = a*syn[2i-1]   + h[2i]     (STT)
The mem recurrence repeats the same pyramid over syn16 with beta.
"""

import os
from contextlib import ExitStack

import numpy as np

import concourse.bass as bass
import concourse.tile as tile
from concourse import bacc, mybir
from concourse import bass_utils

B, T, C, D = 512, 100, 700, 512
NCORES = 8
BC = B // NCORES  # 64 batches per core
NQ = 4  # quartets: 16 batches = 1600 columns each
QCOLS = 1600
KT = [(k * 128, min(128, C - k * 128)) for k in range(6)]
# PSUM bank splits of the shifted matmul output: out col = x col + 16,
# banks are 512-wide f32
MM_SPLITS = [(0, 496), (496, 512), (1008, 512), (1520, 64)]  # (x col0, width)
F32 = mybir.dt.float32
FP16 = mybir.dt.float16
MULT = mybir.AluOpType.mult
ADD = mybir.AluOpType.add

MODE = "fp16-blelloch-tmajor"
LAST_RESULT = None
_cache = {}


def _build():
    if "nc" in _cache:
        return _cache["nc"]
    nc = bacc.Bacc("TRN2", target_bir_lowering=False, debug=False)

    xt_d = nc.dram_tensor("xt16", [C, BC * T], FP16, kind="ExternalInput").ap()
    w_d = nc.dram_tensor("w16", [C, D], FP16, kind="ExternalInput").ap()
    pows_d = nc.dram_tensor("pows", [128, 24], F32, kind="ExternalInput").ap()
    syn_d = nc.dram_tensor("syn", [D, BC * T], FP16, kind="ExternalOutput").ap()
    mem_d = nc.dram_tensor("mem", [D, BC * T], FP16, kind="ExternalOutput").ap()

    with tile.TileContext(nc) as tc:
        with ExitStack() as ctx:
            cpool = ctx.enter_context(tc.tile_pool(name="consts", bufs=1))
            # weights + coef powers on the gpsimd (SWDGE) queue so the Sync
            # queue leads with the first x tiles
            w_tiles = []
            for k, (r0, rk) in enumerate(KT):
                wt = cpool.tile([128, D], FP16, name=f"w{k}", tag=f"w{k}")
                nc.gpsimd.dma_start(wt[:rk, :], w_d[r0 : r0 + rk, :])
                w_tiles.append(wt)
            pows = cpool.tile([128, 24], F32, name="pows", tag="pows")
            nc.gpsimd.dma_start(pows[:], pows_d)

            # quad-scan coef tiles (batch-major [j,u]): a^4 (resp b^4)
            # everywhere, 0 at u=0 (scan reset); generated on device
            qa, qb = [], []
            for di in range(4):
                ta = cpool.tile([128, 400], F32, name=f"qa{di}", tag=f"qa{di}")
                nc.vector.tensor_scalar_mul(
                    ta[:], pows[:, di * 6 + 2 : di * 6 + 3].broadcast_to((128, 400)), 1.0
                )
                nc.vector.memset(ta[:, 0::25], 0.0)
                qa.append(ta)
                tb = cpool.tile([128, 400], F32, name=f"qb{di}", tag=f"qb{di}")
                nc.vector.tensor_scalar_mul(
                    tb[:], pows[:, di * 6 + 5 : di * 6 + 6].broadcast_to((128, 400)), 1.0
                )
                nc.vector.memset(tb[:, 0::25], 0.0)
                qb.append(tb)

            xp = ctx.enter_context(tc.tile_pool(name="xp", bufs=2))
            pp = ctx.enter_context(tc.tile_pool(name="pp", bufs=2, space="PSUM"))
            sp = ctx.enter_context(tc.tile_pool(name="sp", bufs=2))
            vp = ctx.enter_context(tc.tile_pool(name="vp", bufs=2))
            gp = ctx.enter_context(tc.tile_pool(name="gp", bufs=1))

            # PE warmup: dummy matmuls run during the initial DMA wait so HAM
            # un-throttles before the first real MM
            warm_sb = cpool.tile([128, 512], FP16, name="warm", tag="warm")
            nc.vector.memset(warm_sb[:], 0.0)
            warm_ps = pp.tile([128, 2048], F32, tag="ps", name="warm_ps")
            for _ in range(16):
                nc.tensor.matmul(
                    warm_ps[:, 0:512], warm_sb[:, 0:128], warm_sb[:], start=True, stop=True
                )

            for q in range(NQ):
                qc0 = q * QCOLS
                xts = []
                for k, (r0, rk) in enumerate(KT):
                    t_ = xp.tile([128, QCOLS], FP16, tag=f"x{k}", name=f"x{k}_{q}")
                    nc.sync.dma_start(t_[:rk, :], xt_d[r0 : r0 + rk, qc0 : qc0 + QCOLS])
                    xts.append(t_)

                for di in range(4):
                    dsl = slice(di * 128, (di + 1) * 128)
                    j6 = di * 6
                    aS = pows[:, j6 + 0 : j6 + 1]
                    a2S = pows[:, j6 + 1 : j6 + 2]
                    bS = pows[:, j6 + 3 : j6 + 4]
                    b2S = pows[:, j6 + 4 : j6 + 5]

                    # h matmul: shifted write (+16 cols), 4 bank-aligned splits
                    ps = pp.tile([128, 2048], F32, tag="ps", name=f"ps_{q}_{di}")
                    for k, (r0, rk) in enumerate(KT):
                        lhsT = w_tiles[k][:rk, dsl]
                        for c0, wid in MM_SPLITS:
                            nc.tensor.matmul(
                                ps[:, c0 + 16 : c0 + 16 + wid],
                                lhsT,
                                xts[k][:rk, c0 : c0 + wid],
                                start=(k == 0),
                                stop=(k == 5),
                            )
                    nc.vector.memset(ps[:, 0:16], 0.0)  # h[t=0] := 0

                    syn16 = sp.tile([128, QCOLS], FP16, tag="syn", name=f"sy_{q}_{di}")
                    v16 = vp.tile([128, QCOLS], FP16, tag="v", name=f"v_{q}_{di}")
                    g1 = gp.tile([128, 800], F32, tag="g1", name=f"g1_{q}_{di}")
                    g2 = gp.tile([128, 400], F32, tag="g2", name=f"g2_{q}_{di}")
                    s4 = gp.tile([128, 400], F32, tag="s4", name=f"s4_{q}_{di}")
                    g1m = gp.tile([128, 800], F32, tag="g1m", name=f"g1m_{q}_{di}")
                    g2m = gp.tile([128, 400], F32, tag="g2m", name=f"g2m_{q}_{di}")
                    s4m = gp.tile([128, 400], F32, tag="s4m", name=f"s4m_{q}_{di}")

                    # [p, i, j] views: cols off + 32*i + j
                    ps_i = ps.rearrange("p (i r) -> p i r", r=32)
                    sy_i = syn16.rearrange("p (i r) -> p i r", r=32)
                    v_i = v16.rearrange("p (i r) -> p i r", r=32)
                    g1_i = g1.rearrange("p (i j) -> p i j", j=16)
                    # [p, j, u] batch-major views of g1 (cols 32u + j + off)
                    g1_ju = g1.rearrange("p (u j) -> p j u", j=16)
                    g1m_i = g1m.rearrange("p (i j) -> p i j", j=16)
                    g1m_ju = g1m.rearrange("p (u j) -> p j u", j=16)
                    # [p, u, j] / [p, j, u] views with stride 64 (cols 64u+off+j)
                    sy_u = syn16.rearrange("p (u r) -> p u r", r=64)
                    v_u = v16.rearrange("p (u r) -> p u r", r=64)
                    g2_ju = g2.rearrange("p (j u) -> p j u", u=25)
                    g2m_ju = g2m.rearrange("p (j u) -> p j u", u=25)
                    s4_ju = s4.rearrange("p (j u) -> p j u", u=25)
                    s4m_ju = s4m.rearrange("p (j u) -> p j u", u=25)

                    # ---- syn pyramid ----
                    # pair-combine reads two PSUM operands, which the DVE
                    # can't; the ACT engine applies the alpha scale (PSUM ->
                    # SBUF), then the DVE add has a single PSUM input
                    nc.scalar.activation(
                        g1_i, ps_i[:, 0:50, 0:16],
                        mybir.ActivationFunctionType.Copy, scale=aS,
                    )
                    nc.vector.tensor_tensor(
                        g1_i, g1_i, ps_i[:, 0:50, 16:32], ADD
                    )
                    nc.vector.scalar_tensor_tensor(
                        g2_ju, g1_ju[:, :, 0::2], a2S, g1_ju[:, :, 1::2], MULT, ADD
                    )
                    nc.vector.tensor_tensor_scan(s4[:], qa[di][:], g2[:], 0.0, MULT, ADD)
                    # s4 -> syn16 at t=4u+3 (cols 64u+48+j)
                    nc.vector.tensor_scalar_mul(
                        sy_u[:, :, 48:64].rearrange("p u j -> p j u"), s4_ju, 1.0
                    )
                    # t=1 (cols 16+j) = g1[i=0]
                    nc.vector.tensor_scalar_mul(syn16[:, 16:32], g1[:, 0:16], 1.0)
                    # t=4u+1, u>=1 (cols 64u+16+j) = a2*syn[4u-1] + g1[2u]
                    nc.vector.scalar_tensor_tensor(
                        sy_u[:, 1:25, 16:32], sy_u[:, 0:24, 48:64], a2S,
                        g1_i[:, 2:50:2, :], MULT, ADD,
                    )
                    nc.vector.memset(syn16[:, 0:16], 0.0)  # t=0
                    # t=2i, i>=1 (cols 32i+j) = a*syn[2i-1] + h[2i]
                    nc.vector.scalar_tensor_tensor(
                        sy_i[:, 1:50, 0:16], sy_i[:, 0:49, 16:32], aS,
                        ps_i[:, 1:50, 0:16], MULT, ADD,
                    )
                    nc.scalar.dma_start(syn_d[dsl, qc0 : qc0 + QCOLS], syn16[:])

                    # ---- mem pyramid (over syn16; u = mem/(1-beta), scaled on
                    # host; stored shifted +16 cols = one time step) ----
                    nc.vector.scalar_tensor_tensor(
                        g1m_i, sy_i[:, 0:50, 0:16], bS, sy_i[:, 0:50, 16:32], MULT, ADD
                    )
                    nc.vector.scalar_tensor_tensor(
                        g2m_ju, g1m_ju[:, :, 0::2], b2S, g1m_ju[:, :, 1::2], MULT, ADD
                    )
                    nc.vector.tensor_tensor_scan(s4m[:], qb[di][:], g2m[:], 0.0, MULT, ADD)
                    nc.vector.tensor_scalar_mul(
                        v_u[:, :, 48:64].rearrange("p u j -> p j u"), s4m_ju, 1.0
                    )
                    nc.vector.tensor_scalar_mul(v16[:, 16:32], g1m[:, 0:16], 1.0)
                    nc.vector.scalar_tensor_tensor(
                        v_u[:, 1:25, 16:32], v_u[:, 0:24, 48:64], b2S,
                        g1m_i[:, 2:50:2, :], MULT, ADD,
                    )
                    nc.vector.memset(v16[:, 0:16], 0.0)
                    nc.vector.scalar_tensor_tensor(
                        v_i[:, 1:50, 0:16], v_i[:, 0:49, 16:32], bS,
                        sy_i[:, 1:50, 0:16], MULT, ADD,
                    )
                    nc.scalar.dma_start(
                        mem_d[dsl, qc0 + 16 : qc0 + QCOLS], v16[:, 0 : QCOLS - 16]
                    )

    nc.compile()
    _cache["nc"] = nc
    return nc


def kernel(inputs, w, alpha, beta):
    global LAST_RESULT
    inputs = np.asarray(inputs, dtype=np.float32)
    w = np.asarray(w, dtype=np.float32)
    alpha = np.asarray(alpha, dtype=np.float32).reshape(-1)
    beta = np.asarray(beta, dtype=np.float32).reshape(-1)

    nc = _build()

    a2 = alpha * alpha
    b2 = beta * beta
    pows = np.zeros((128, 24), dtype=np.float32)
    for di in range(4):
        sl = slice(di * 128, (di + 1) * 128)
        pows[:, di * 6 + 0] = alpha[sl]
        pows[:, di * 6 + 1] = a2[sl]
        pows[:, di * 6 + 2] = a2[sl] * a2[sl]
        pows[:, di * 6 + 3] = beta[sl]
        pows[:, di * 6 + 4] = b2[sl]
        pows[:, di * 6 + 5] = b2[sl] * b2[sl]
    w16 = w.astype(np.float16)
    omb_col = (1.0 - beta).reshape(D, 1)

    in_maps = []
    for c in range(NCORES):
        xc = inputs[c * BC : (c + 1) * BC]  # [64, 100, 700]
        # t-major interleave: col = q*1600 + 16*t + j
        xt16 = (
            xc.reshape(NQ, 16, T, C).transpose(3, 0, 2, 1).reshape(C, BC * T)
        ).astype(np.float16)
        in_maps.append({"xt16": xt16, "w16": w16, "pows": pows})

    run_kwargs = {}
    if os.environ.get("MEMBRANE_TRACE_DIR"):
        run_kwargs["tmpdir"] = os.environ["MEMBRANE_TRACE_DIR"]
    res = bass_utils.run_bass_kernel_spmd(
        nc, in_maps, core_ids=list(range(NCORES)), **run_kwargs
    )
    LAST_RESULT = res

    syn_full = np.empty((B, T, D), dtype=np.float32)
    mem_full = np.empty((B, T, D), dtype=np.float32)
    for c in range(NCORES):
        r = res.results[c]
        cs = slice(c * BC, (c + 1) * BC)
        syn_full[cs] = (
            r["syn"].astype(np.float32).reshape(D, NQ, T, 16)
            .transpose(1, 3, 2, 0).reshape(BC, T, D)
        )
        mem_full[cs] = (
            (r["mem"].astype(np.float32) * omb_col).reshape(D, NQ, T, 16)
            .transpose(1, 3, 2, 0).reshape(BC, T, D)
        )
    syn_full[:, 0, :] = 0.0
    mem_full[:, 0, :] = 0.0
    return (syn_full, mem_full)


# revision 11
# speedup vs baseline: 1.1700x; 1.1700x over previous
"""Trainium2 Bass kernel for nn_MembraneLayer: h = x @ w followed by a
double first-order recurrence over time, producing (syn_rec, mem_rec).

Sharding: data-parallel over batch. 8 cores x 64 batches each.

Layout (hardcoded): per quartet of 16 batches, columns are sorted by
t mod 4 into four REGIONS, batch-major (j, u) within each region
(t = 4u + r). Region starts are PSUM-bank aligned, so the shifted
matmul write (slot t+1), every Blelloch level, and the serial scan all
operate on fully CONTIGUOUS 400-column blocks (strided DVE access runs
at half rate; contiguous runs at full rate). The host does the mod-4
gather on x and the inverse permutation + one-step shift on the
outputs.

  xt16 [700, 6336] fp16   x gathered to [C, (q, r-block, j, u)]
  w16  [700, 512]  fp16
  pows [128, 24]   f32    per (partition, d_tile): alpha, a^2, a^4, beta,
                          b^2, b^4
  syn  [512, 6400] fp16   syn_rec slots in region layout
  mem  [512, 6400] fp16   v = mem/(1-beta) slots in region layout;
                          host applies (1-beta) and the t+1 shift

Per (quartet, d_tile): single-pass fp16 matmuls accumulate h into PSUM
regions r1..r3 (contiguous) and r0 (rank-3, slots 4u at j*25+u). The
K=4 Blelloch block-scan (serial scan only on 1/4 of columns; the rest
are full-rate STT/ACT ops with per-partition scalars):
  ACT   te = a*h[4u],  to = a*h[4u+2]     (PSUM->SBUF, scalar engine)
  DVE   P  = te + h[4u+1]; Q = to + h[4u+3]
        G  = a^2*P + Q
        S3 = serial scan of G, coef a^4    -> syn[4u+3]  (region 3)
        S1 = a^2*S3[u-1] + P               -> syn[4u+1]
        S2 = a*S1 + h[4u+2]                -> syn[4u+2]
        S0 = a*S3[u-1] + h[4u]             -> syn[4u]
The mem recurrence repeats the same pyramid over syn16 with beta (pure
DVE STT; inputs are SBUF).
"""

import os
from contextlib import ExitStack

import numpy as np

import concourse.bass as bass
import concourse.tile as tile
from concourse import bacc, mybir
from concourse import bass_utils

B, T, C, D = 512, 100, 700, 512
NCORES = 8
BC = B // NCORES  # 64 batches per core
NQ = 4  # quartets: 16 batches each
XQ = 1584  # x cols per quartet: 3*400 + 384 (t=99 never used)
SQ = 1600  # output cols per quartet: 4 regions x 400
KT = [(k * 128, min(128, C - k * 128)) for k in range(6)]
F32 = mybir.dt.float32
FP16 = mybir.dt.float16
MULT = mybir.AluOpType.mult
ADD = mybir.AluOpType.add
COPY = mybir.ActivationFunctionType.Copy

MODE = "fp16-blelloch-regions"
LAST_RESULT = None
_cache = {}


def _build(sim_safe=False):
    """sim_safe=True splits the rank-3 r0 matmul per batch so CoreSim's
    2-D result assert passes; numerics identical to the HW build."""
    key = ("nc", sim_safe)
    if key in _cache:
        return _cache[key]
    nc = bacc.Bacc("TRN2", target_bir_lowering=False, debug=False)

    xt_d = nc.dram_tensor("xt16", [C, NQ * XQ], FP16, kind="ExternalInput").ap()
    w_d = nc.dram_tensor("w16", [C, D], FP16, kind="ExternalInput").ap()
    pows_d = nc.dram_tensor("pows", [128, 24], F32, kind="ExternalInput").ap()
    syn_d = nc.dram_tensor("syn", [D, NQ * SQ], FP16, kind="ExternalOutput").ap()
    mem_d = nc.dram_tensor("mem", [D, NQ * SQ], FP16, kind="ExternalOutput").ap()

    with tile.TileContext(nc) as tc:
        with ExitStack() as ctx:
            cpool = ctx.enter_context(tc.tile_pool(name="consts", bufs=1))
            # weights + coef powers on the gpsimd (SWDGE) queue so the Sync
            # queue leads with the first x tiles
            w_tiles = []
            for k, (r0_, rk) in enumerate(KT):
                wt = cpool.tile([128, D], FP16, name=f"w{k}", tag=f"w{k}")
                nc.gpsimd.dma_start(wt[:rk, :], w_d[r0_ : r0_ + rk, :])
                w_tiles.append(wt)
            pows = cpool.tile([128, 24], F32, name="pows", tag="pows")
            nc.gpsimd.dma_start(pows[:], pows_d)

            # quad-scan coef tiles (batch-major [j,u]): a^4 (resp b^4)
            # everywhere, 0 at u=0 (per-batch scan reset); generated on device
            qa, qb = [], []
            for di in range(4):
                ta = cpool.tile([128, 400], F32, name=f"qa{di}", tag=f"qa{di}")
                nc.vector.tensor_scalar_mul(
                    ta[:], pows[:, di * 6 + 2 : di * 6 + 3].broadcast_to((128, 400)), 1.0
                )
                nc.vector.memset(ta[:, 0::25], 0.0)
                qa.append(ta)
                tb = cpool.tile([128, 400], F32, name=f"qb{di}", tag=f"qb{di}")
                nc.vector.tensor_scalar_mul(
                    tb[:], pows[:, di * 6 + 5 : di * 6 + 6].broadcast_to((128, 400)), 1.0
                )
                nc.vector.memset(tb[:, 0::25], 0.0)
                qb.append(tb)

            xp = ctx.enter_context(tc.tile_pool(name="xp", bufs=2))
            pp = ctx.enter_context(tc.tile_pool(name="pp", bufs=2, space="PSUM"))
            sp = ctx.enter_context(tc.tile_pool(name="sp", bufs=2))
            vp = ctx.enter_context(tc.tile_pool(name="vp", bufs=2))
            gp = ctx.enter_context(tc.tile_pool(name="gp", bufs=1))

            # PE warmup: dummy matmuls run during the initial DMA wait so HAM
            # un-throttles before the first real MM
            warm_sb = cpool.tile([128, 512], FP16, name="warm", tag="warm")
            nc.vector.memset(warm_sb[:], 0.0)
            warm_ps = pp.tile([128, 2048], F32, tag="ps", name="warm_ps")
            for _ in range(16):
                nc.tensor.matmul(
                    warm_ps[:, 0:512], warm_sb[:, 0:128], warm_sb[:], start=True, stop=True
                )

            for q in range(NQ):
                xc0 = q * XQ
                sc0 = q * SQ
                xts = []
                for k, (r0_, rk) in enumerate(KT):
                    t_ = xp.tile([128, XQ], FP16, tag=f"x{k}", name=f"x{k}_{q}")
                    nc.sync.dma_start(t_[:rk, :], xt_d[r0_ : r0_ + rk, xc0 : xc0 + XQ])
                    xts.append(t_)

                for di in range(4):
                    dsl = slice(di * 128, (di + 1) * 128)
                    j6 = di * 6
                    aS = pows[:, j6 + 0 : j6 + 1]
                    a2S = pows[:, j6 + 1 : j6 + 2]
                    bS = pows[:, j6 + 3 : j6 + 4]
                    b2S = pows[:, j6 + 4 : j6 + 5]

                    # h matmul into PSUM regions (bank-aligned starts)
                    ps = pp.tile([128, 2048], F32, tag="ps", name=f"ps_{q}_{di}")
                    hR = [ps[:, 0:400], ps[:, 512:912], ps[:, 1024:1424], ps[:, 1536:1936]]
                    hR0v = hR[0].rearrange("p (j u) -> p j u", u=25)
                    for k, (r0_, rk) in enumerate(KT):
                        lhsT = w_tiles[k][:rk, dsl]
                        # x blocks [0:400)->r1, [400:800)->r2, [800:1200)->r3
                        for bi, reg in ((0, 1), (1, 2), (2, 3)):
                            nc.tensor.matmul(
                                hR[reg], lhsT, xts[k][:rk, bi * 400 : bi * 400 + 400],
                                start=(k == 0), stop=(k == 5),
                            )
                        # x block [1200:1584) -> r0 slots u=1..24
                        if sim_safe:
                            for j in range(16):
                                nc.tensor.matmul(
                                    ps[:, j * 25 + 1 : j * 25 + 25], lhsT,
                                    xts[k][:rk, 1200 + j * 24 : 1200 + (j + 1) * 24],
                                    start=(k == 0 and j == 0),
                                    stop=(k == 5 and j == 15),
                                )
                        else:
                            nc.tensor.matmul(
                                hR0v[:, :, 1:25], lhsT,
                                xts[k][:rk, 1200:1584].rearrange("p (j u) -> p j u", u=24),
                                start=(k == 0), stop=(k == 5),
                            )
                    nc.vector.memset(hR0v[:, :, 0:1], 0.0)  # h[t=0] := 0

                    syn16 = sp.tile([128, SQ], FP16, tag="syn", name=f"sy_{q}_{di}")
                    v16 = vp.tile([128, SQ], FP16, tag="v", name=f"v_{q}_{di}")
                    S = [syn16[:, r * 400 : (r + 1) * 400] for r in range(4)]
                    V = [v16[:, r * 400 : (r + 1) * 400] for r in range(4)]
                    Sv = [x_.rearrange("p (j u) -> p j u", u=25) for x_ in S]
                    Vv = [x_.rearrange("p (j u) -> p j u", u=25) for x_ in V]

                    te = gp.tile([128, 400], F32, tag="te", name=f"te_{q}_{di}")
                    to = gp.tile([128, 400], F32, tag="to", name=f"to_{q}_{di}")
                    P = gp.tile([128, 400], F32, tag="P", name=f"P_{q}_{di}")
                    Q = gp.tile([128, 400], F32, tag="Q", name=f"Q_{q}_{di}")
                    G = gp.tile([128, 400], F32, tag="G", name=f"G_{q}_{di}")
                    Pv = P.rearrange("p (j u) -> p j u", u=25)

                    # ---- syn pyramid ----
                    # the pair-combines need two PSUM operands; ACT applies the
                    # alpha scale (PSUM->SBUF) so each DVE add has one
                    nc.scalar.activation(te[:], hR[0], COPY, scale=aS)
                    nc.scalar.activation(to[:], hR[2], COPY, scale=aS)
                    nc.vector.tensor_tensor(P[:], te[:], hR[1], ADD)
                    nc.vector.tensor_tensor(Q[:], to[:], hR[3], ADD)
                    nc.vector.scalar_tensor_tensor(G[:], P[:], a2S, Q[:], MULT, ADD)
                    nc.vector.tensor_tensor_scan(S[3], qa[di][:], G[:], 0.0, MULT, ADD)
                    # syn[4u+1] = a^2*syn[4u-1] + P[u]
                    nc.vector.scalar_tensor_tensor(
                        Sv[1][:, :, 1:25], Sv[3][:, :, 0:24], a2S, Pv[:, :, 1:25],
                        MULT, ADD,
                    )
                    nc.vector.tensor_scalar_mul(Sv[1][:, :, 0:1], Pv[:, :, 0:1], 1.0)
                    # syn[4u+2] = a*syn[4u+1] + h[4u+2]
                    nc.vector.scalar_tensor_tensor(S[2], S[1], aS, hR[2], MULT, ADD)
                    # syn[4u+4] = a*syn[4u+3] + h[4u+4]
                    nc.vector.scalar_tensor_tensor(
                        Sv[0][:, :, 1:25], Sv[3][:, :, 0:24], aS, hR0v[:, :, 1:25],
                        MULT, ADD,
                    )
                    nc.vector.memset(Sv[0][:, :, 0:1], 0.0)  # t=0
                    nc.scalar.dma_start(syn_d[dsl, sc0 : sc0 + SQ], syn16[:])

                    # ---- mem pyramid over syn16 (all-SBUF; u = mem/(1-beta),
                    # scaled + shifted one step on the host) ----
                    Pm = gp.tile([128, 400], F32, tag="Pm", name=f"Pm_{q}_{di}")
                    Qm = gp.tile([128, 400], F32, tag="Qm", name=f"Qm_{q}_{di}")
                    Gm = gp.tile([128, 400], F32, tag="Gm", name=f"Gm_{q}_{di}")
                    Pmv = Pm.rearrange("p (j u) -> p j u", u=25)

                    nc.vector.scalar_tensor_tensor(Pm[:], S[0], bS, S[1], MULT, ADD)
                    nc.vector.scalar_tensor_tensor(Qm[:], S[2], bS, S[3], MULT, ADD)
                    nc.vector.scalar_tensor_tensor(Gm[:], Pm[:], b2S, Qm[:], MULT, ADD)
                    nc.vector.tensor_tensor_scan(V[3], qb[di][:], Gm[:], 0.0, MULT, ADD)
                    nc.vector.scalar_tensor_tensor(
                        Vv[1][:, :, 1:25], Vv[3][:, :, 0:24], b2S, Pmv[:, :, 1:25],
                        MULT, ADD,
                    )
                    nc.vector.tensor_scalar_mul(Vv[1][:, :, 0:1], Pmv[:, :, 0:1], 1.0)
                    nc.vector.scalar_tensor_tensor(V[2], V[1], bS, S[2], MULT, ADD)
                    nc.vector.scalar_tensor_tensor(
                        Vv[0][:, :, 1:25], Vv[3][:, :, 0:24], bS, Sv[0][:, :, 1:25],
                        MULT, ADD,
                    )
                    nc.vector.memset(Vv[0][:, :, 0:1], 0.0)
                    nc.scalar.dma_start(mem_d[dsl, sc0 : sc0 + SQ], v16[:])

    nc.compile()
    _cache[key] = nc
    return nc


def _prep_x(xc):
    """[64, 100, 700] -> [700, 6336] region-gathered fp16."""
    xe = xc.reshape(NQ, 16, 25, 4, C)  # (q, j, u, r, c); t = 4u + r
    blocks = [
        xe[:, :, :, 0, :].reshape(NQ, 400, C),
        xe[:, :, :, 1, :].reshape(NQ, 400, C),
        xe[:, :, :, 2, :].reshape(NQ, 400, C),
        xe[:, :, 0:24, 3, :].reshape(NQ, 384, C),
    ]
    xb = np.concatenate(blocks, axis=1)  # [NQ, 1584, C]
    return np.ascontiguousarray(xb.transpose(2, 0, 1).reshape(C, NQ * XQ)).astype(
        np.float16
    )


def _unshard(raw):
    """[D, 6400] region layout -> [BC, T, D] (raw slot t values)."""
    return (
        raw.reshape(D, NQ, 4, 16, 25)  # (d, q, r, j, u)
        .transpose(1, 3, 4, 2, 0)      # (q, j, u, r, d);  t = 4u + r
        .reshape(BC, T, D)
    )


def kernel(inputs, w, alpha, beta):
    global LAST_RESULT
    inputs = np.asarray(inputs, dtype=np.float32)
    w = np.asarray(w, dtype=np.float32)
    alpha = np.asarray(alpha, dtype=np.float32).reshape(-1)
    beta = np.asarray(beta, dtype=np.float32).reshape(-1)

    nc = _build()

    a2 = alpha * alpha
    b2 = beta * beta
    pows = np.zeros((128, 24), dtype=np.float32)
    for di in range(4):
        sl = slice(di * 128, (di + 1) * 128)
        pows[:, di * 6 + 0] = alpha[sl]
        pows[:, di * 6 + 1] = a2[sl]
        pows[:, di * 6 + 2] = a2[sl] * a2[sl]
        pows[:, di * 6 + 3] = beta[sl]
        pows[:, di * 6 + 4] = b2[sl]
        pows[:, di * 6 + 5] = b2[sl] * b2[sl]
    w16 = w.astype(np.float16)
    omb = (1.0 - beta).reshape(1, 1, D)

    in_maps = []
    for c in range(NCORES):
        in_maps.append(
            {"xt16": _prep_x(inputs[c * BC : (c + 1) * BC]), "w16": w16, "pows": pows}
        )

    run_kwargs = {}
    if os.environ.get("MEMBRANE_TRACE_DIR"):
        run_kwargs["tmpdir"] = os.environ["MEMBRANE_TRACE_DIR"]
    res = bass_utils.run_bass_kernel_spmd(
        nc, in_maps, core_ids=list(range(NCORES)), **run_kwargs
    )
    LAST_RESULT = res

    syn_full = np.empty((B, T, D), dtype=np.float32)
    mem_full = np.empty((B, T, D), dtype=np.float32)
    for c in range(NCORES):
        r = res.results[c]
        cs = slice(c * BC, (c + 1) * BC)
        syn_full[cs] = _unshard(r["syn"].astype(np.float32))
        vt = _unshard(r["mem"].astype(np.float32))
        mem_full[cs, 1:, :] = vt[:, : T - 1, :] * omb
    syn_full[:, 0, :] = 0.0
    mem_full[:, 0, :] = 0.0
    return (syn_full, mem_full)


# revision 13
# speedup vs baseline: 1.1883x; 1.0157x over previous
"""Trainium2 Bass kernel for nn_MembraneLayer: h = x @ w followed by a
double first-order recurrence over time, producing (syn_rec, mem_rec).

Sharding: data-parallel over batch. 8 cores x 64 batches each.

Layout (hardcoded): per quartet of 16 batches, columns are sorted by
t mod 4 into four REGIONS, batch-major (j, u) within each region
(t = 4u + r). Region starts are PSUM-bank aligned, so the shifted
matmul write (slot t+1), every Blelloch level, and the serial scan all
operate on fully CONTIGUOUS 400-column blocks (strided DVE access runs
at half rate; contiguous runs at full rate). The host does the mod-4
gather on x and the inverse permutation + one-step shift on the
outputs.

  xt16 [700, 6336] fp16   x gathered to [C, (q, r-block, j, u)]
  w16  [700, 512]  fp16
  pows [128, 24]   f32    per (partition, d_tile): alpha, a^2, a^4, beta,
                          b^2, b^4
  syn  [512, 6400] fp16   syn_rec slots in region layout
  mem  [512, 6400] fp16   v = mem/(1-beta) slots in region layout;
                          host applies (1-beta) and the t+1 shift

Per (quartet, d_tile): single-pass fp16 matmuls accumulate h into PSUM
regions r1..r3 (contiguous) and r0 (rank-3, slots 4u at j*25+u). The
K=4 Blelloch block-scan (serial scan only on 1/4 of columns; the rest
are full-rate STT/ACT ops with per-partition scalars):
  ACT   te = a*h[4u],  to = a*h[4u+2]     (PSUM->SBUF, scalar engine)
  DVE   P  = te + h[4u+1]; Q = to + h[4u+3]
        G  = a^2*P + Q
        S3 = serial scan of G, coef a^4    -> syn[4u+3]  (region 3)
        S1 = a^2*S3[u-1] + P               -> syn[4u+1]
        S2 = a*S1 + h[4u+2]                -> syn[4u+2]
        S0 = a*S3[u-1] + h[4u]             -> syn[4u]
The mem recurrence repeats the same pyramid over syn16 with beta (pure
DVE STT; inputs are SBUF).
"""

import os
from contextlib import ExitStack

import numpy as np

import concourse.bass as bass
import concourse.tile as tile
from concourse import bacc, mybir
from concourse import bass_utils

B, T, C, D = 512, 100, 700, 512
NCORES = 8
BC = B // NCORES  # 64 batches per core
NQ = 4  # quartets: 16 batches each
XQ = 1584  # x cols per quartet: 3*400 + 384 (t=99 never used)
SQ = 1600  # output cols per quartet: 4 regions x 400
KT = [(k * 128, min(128, C - k * 128)) for k in range(6)]
F32 = mybir.dt.float32
FP16 = mybir.dt.float16
MULT = mybir.AluOpType.mult
ADD = mybir.AluOpType.add
COPY = mybir.ActivationFunctionType.Copy

MODE = "fp16-blelloch-regions"
LAST_RESULT = None
_cache = {}


def _build(sim_safe=False):
    """sim_safe=True splits the rank-3 r0 matmul per batch so CoreSim's
    2-D result assert passes; numerics identical to the HW build."""
    key = ("nc", sim_safe)
    if key in _cache:
        return _cache[key]
    nc = bacc.Bacc("TRN2", target_bir_lowering=False, debug=False)

    xt_d = nc.dram_tensor("xt16", [C, NQ * XQ], FP16, kind="ExternalInput").ap()
    w_d = nc.dram_tensor("w16", [C, D], FP16, kind="ExternalInput").ap()
    pows_d = nc.dram_tensor("pows", [128, 24], F32, kind="ExternalInput").ap()
    syn_d = nc.dram_tensor("syn", [D, NQ * SQ], FP16, kind="ExternalOutput").ap()
    mem_d = nc.dram_tensor("mem", [D, NQ * SQ], FP16, kind="ExternalOutput").ap()

    with tile.TileContext(nc) as tc:
        with ExitStack() as ctx:
            cpool = ctx.enter_context(tc.tile_pool(name="consts", bufs=1))
            # weights + coef powers on the gpsimd (SWDGE) queue so the Sync
            # queue leads with the first x tiles
            w_tiles = []
            for k, (r0_, rk) in enumerate(KT):
                wt = cpool.tile([128, D], FP16, name=f"w{k}", tag=f"w{k}")
                nc.gpsimd.dma_start(wt[:rk, :], w_d[r0_ : r0_ + rk, :])
                w_tiles.append(wt)
            pows = cpool.tile([128, 24], F32, name="pows", tag="pows")
            nc.gpsimd.dma_start(pows[:], pows_d)

            # quad-scan coef tiles (batch-major [j,u]): a^4 (resp b^4)
            # everywhere, 0 at u=0 (per-batch scan reset); generated on device
            qa, qb = [], []
            for di in range(4):
                ta = cpool.tile([128, 400], F32, name=f"qa{di}", tag=f"qa{di}")
                nc.vector.tensor_scalar_mul(
                    ta[:], pows[:, di * 6 + 2 : di * 6 + 3].broadcast_to((128, 400)), 1.0
                )
                nc.vector.memset(ta[:, 0::25], 0.0)
                qa.append(ta)
                tb = cpool.tile([128, 400], F32, name=f"qb{di}", tag=f"qb{di}")
                nc.vector.tensor_scalar_mul(
                    tb[:], pows[:, di * 6 + 5 : di * 6 + 6].broadcast_to((128, 400)), 1.0
                )
                nc.vector.memset(tb[:, 0::25], 0.0)
                qb.append(tb)

            xp = ctx.enter_context(tc.tile_pool(name="xp", bufs=2))
            pp = ctx.enter_context(tc.tile_pool(name="pp", bufs=2, space="PSUM"))
            sp = ctx.enter_context(tc.tile_pool(name="sp", bufs=2))
            vp = ctx.enter_context(tc.tile_pool(name="vp", bufs=2))
            gp = ctx.enter_context(tc.tile_pool(name="gp", bufs=1))

            # PE warmup: dummy matmuls run during the initial DMA wait so HAM
            # un-throttles before the first real MM
            warm_sb = cpool.tile([128, 512], FP16, name="warm", tag="warm")
            nc.vector.memset(warm_sb[:], 0.0)
            warm_ps = pp.tile([128, 2048], F32, tag="ps", name="warm_ps")
            for _ in range(16):
                nc.tensor.matmul(
                    warm_ps[:, 0:512], warm_sb[:, 0:128], warm_sb[:], start=True, stop=True
                )

            for q in range(NQ):
                xc0 = q * XQ
                sc0 = q * SQ
                xts = []
                for k, (r0_, rk) in enumerate(KT):
                    t_ = xp.tile([128, XQ], FP16, tag=f"x{k}", name=f"x{k}_{q}")
                    nc.sync.dma_start(t_[:rk, :], xt_d[r0_ : r0_ + rk, xc0 : xc0 + XQ])
                    xts.append(t_)

                for di in range(4):
                    dsl = slice(di * 128, (di + 1) * 128)
                    j6 = di * 6
                    aS = pows[:, j6 + 0 : j6 + 1]
                    a2S = pows[:, j6 + 1 : j6 + 2]
                    bS = pows[:, j6 + 3 : j6 + 4]
                    b2S = pows[:, j6 + 4 : j6 + 5]

                    # h matmul into PSUM regions (bank-aligned starts)
                    ps = pp.tile([128, 2048], F32, tag="ps", name=f"ps_{q}_{di}")
                    hR = [ps[:, 0:400], ps[:, 512:912], ps[:, 1024:1424], ps[:, 1536:1936]]
                    hR0v = hR[0].rearrange("p (j u) -> p j u", u=25)
                    for k, (r0_, rk) in enumerate(KT):
                        lhsT = w_tiles[k][:rk, dsl]
                        # x blocks [0:400)->r1, [400:800)->r2, [800:1200)->r3
                        for bi, reg in ((0, 1), (1, 2), (2, 3)):
                            nc.tensor.matmul(
                                hR[reg], lhsT, xts[k][:rk, bi * 400 : bi * 400 + 400],
                                start=(k == 0), stop=(k == 5),
                            )
                        # x block [1200:1584) -> r0 slots u=1..24
                        if sim_safe:
                            for j in range(16):
                                nc.tensor.matmul(
                                    ps[:, j * 25 + 1 : j * 25 + 25], lhsT,
                                    xts[k][:rk, 1200 + j * 24 : 1200 + (j + 1) * 24],
                                    start=(k == 0 and j == 0),
                                    stop=(k == 5 and j == 15),
                                )
                        else:
                            nc.tensor.matmul(
                                hR0v[:, :, 1:25], lhsT,
                                xts[k][:rk, 1200:1584].rearrange("p (j u) -> p j u", u=24),
                                start=(k == 0), stop=(k == 5),
                            )
                    nc.vector.memset(hR0v[:, :, 0:1], 0.0)  # h[t=0] := 0

                    syn16 = sp.tile([128, SQ], FP16, tag="syn", name=f"sy_{q}_{di}")
                    v16 = vp.tile([128, SQ], FP16, tag="v", name=f"v_{q}_{di}")
                    S = [syn16[:, r * 400 : (r + 1) * 400] for r in range(4)]
                    V = [v16[:, r * 400 : (r + 1) * 400] for r in range(4)]
                    Sv = [x_.rearrange("p (j u) -> p j u", u=25) for x_ in S]
                    Vv = [x_.rearrange("p (j u) -> p j u", u=25) for x_ in V]

                    # fp16 intermediates: packed 2-byte operands enable the
                    # DVE 2x mode; precision cost ~2^-11 per combine
                    te = gp.tile([128, 400], F32, tag="te", name=f"te_{q}_{di}")
                    to = gp.tile([128, 400], F32, tag="to", name=f"to_{q}_{di}")
                    P = gp.tile([128, 400], FP16, tag="P", name=f"P_{q}_{di}")
                    Q = gp.tile([128, 400], FP16, tag="Q", name=f"Q_{q}_{di}")
                    G = gp.tile([128, 400], FP16, tag="G", name=f"G_{q}_{di}")
                    Pv = P.rearrange("p (j u) -> p j u", u=25)

                    # ---- syn pyramid ----
                    # the pair-combines need two PSUM operands; ACT applies the
                    # alpha scale (PSUM->SBUF) so each DVE add has one
                    nc.scalar.activation(te[:], hR[0], COPY, scale=aS)
                    nc.scalar.activation(to[:], hR[2], COPY, scale=aS)
                    nc.vector.tensor_tensor(P[:], te[:], hR[1], ADD)
                    nc.vector.tensor_tensor(Q[:], to[:], hR[3], ADD)
                    nc.vector.scalar_tensor_tensor(G[:], P[:], a2S, Q[:], MULT, ADD)
                    nc.vector.tensor_tensor_scan(S[3], qa[di][:], G[:], 0.0, MULT, ADD)
                    # syn[4u+1] = a^2*syn[4u-1] + P[u]
                    nc.vector.scalar_tensor_tensor(
                        Sv[1][:, :, 1:25], Sv[3][:, :, 0:24], a2S, Pv[:, :, 1:25],
                        MULT, ADD,
                    )
                    nc.vector.tensor_scalar_mul(Sv[1][:, :, 0:1], Pv[:, :, 0:1], 1.0)
                    # syn[4u+2] = a*syn[4u+1] + h[4u+2]
                    nc.vector.scalar_tensor_tensor(S[2], S[1], aS, hR[2], MULT, ADD)
                    # syn[4u+4] = a*syn[4u+3] + h[4u+4]
                    nc.vector.scalar_tensor_tensor(
                        Sv[0][:, :, 1:25], Sv[3][:, :, 0:24], aS, hR0v[:, :, 1:25],
                        MULT, ADD,
                    )
                    nc.vector.memset(Sv[0][:, :, 0:1], 0.0)  # t=0
                    nc.scalar.dma_start(syn_d[dsl, sc0 : sc0 + SQ], syn16[:])

                    # ---- mem pyramid over syn16 (all-SBUF; u = mem/(1-beta),
                    # scaled + shifted one step on the host) ----
                    Pm = gp.tile([128, 400], FP16, tag="Pm", name=f"Pm_{q}_{di}")
                    Qm = gp.tile([128, 400], FP16, tag="Qm", name=f"Qm_{q}_{di}")
                    Gm = gp.tile([128, 400], FP16, tag="Gm", name=f"Gm_{q}_{di}")
                    Pmv = Pm.rearrange("p (j u) -> p j u", u=25)

                    nc.vector.scalar_tensor_tensor(Pm[:], S[0], bS, S[1], MULT, ADD)
                    nc.vector.scalar_tensor_tensor(Qm[:], S[2], bS, S[3], MULT, ADD)
                    nc.vector.scalar_tensor_tensor(Gm[:], Pm[:], b2S, Qm[:], MULT, ADD)
                    nc.vector.tensor_tensor_scan(V[3], qb[di][:], Gm[:], 0.0, MULT, ADD)
                    nc.vector.scalar_tensor_tensor(
                        Vv[1][:, :, 1:25], Vv[3][:, :, 0:24], b2S, Pmv[:, :, 1:25],
                        MULT, ADD,
                    )
                    nc.vector.tensor_scalar_mul(Vv[1][:, :, 0:1], Pmv[:, :, 0:1], 1.0)
                    nc.vector.scalar_tensor_tensor(V[2], V[1], bS, S[2], MULT, ADD)
                    nc.vector.scalar_tensor_tensor(
                        Vv[0][:, :, 1:25], Vv[3][:, :, 0:24], bS, Sv[0][:, :, 1:25],
                        MULT, ADD,
                    )
                    nc.vector.memset(Vv[0][:, :, 0:1], 0.0)
                    nc.scalar.dma_start(mem_d[dsl, sc0 : sc0 + SQ], v16[:])

    nc.compile()
    _cache[key] = nc
    return nc


def _prep_x(xc):
    """[64, 100, 700] -> [700, 6336] region-gathered fp16."""
    xe = xc.reshape(NQ, 16, 25, 4, C)  # (q, j, u, r, c); t = 4u + r
    blocks = [
        xe[:, :, :, 0, :].reshape(NQ, 400, C),
        xe[:, :, :, 1, :].reshape(NQ, 400, C),
        xe[:, :, :, 2, :].reshape(NQ, 400, C),
        xe[:, :, 0:24, 3, :].reshape(NQ, 384, C),
    ]
    xb = np.concatenate(blocks, axis=1)  # [NQ, 1584, C]
    return np.ascontiguousarray(xb.transpose(2, 0, 1).reshape(C, NQ * XQ)).astype(
        np.float16
    )


def _unshard(raw):
    """[D, 6400] region layout -> [BC, T, D] (raw slot t values)."""
    return (
        raw.reshape(D, NQ, 4, 16, 25)  # (d, q, r, j, u)
        .transpose(1, 3, 4, 2, 0)      # (q, j, u, r, d);  t = 4u + r
        .reshape(BC, T, D)
    )


def kernel(inputs, w, alpha, beta):
    global LAST_RESULT
    inputs = np.asarray(inputs, dtype=np.float32)
    w = np.asarray(w, dtype=np.float32)
    alpha = np.asarray(alpha, dtype=np.float32).reshape(-1)
    beta = np.asarray(beta, dtype=np.float32).reshape(-1)

    nc = _build()

    a2 = alpha * alpha
    b2 = beta * beta
    pows = np.zeros((128, 24), dtype=np.float32)
    for di in range(4):
        sl = slice(di * 128, (di + 1) * 128)
        pows[:, di * 6 + 0] = alpha[sl]
        pows[:, di * 6 + 1] = a2[sl]
        pows[:, di * 6 + 2] = a2[sl] * a2[sl]
        pows[:, di * 6 + 3] = beta[sl]
        pows[:, di * 6 + 4] = b2[sl]
        pows[:, di * 6 + 5] = b2[sl] * b2[sl]
    w16 = w.astype(np.float16)
    omb = (1.0 - beta).reshape(1, 1, D)

    in_maps = []
    for c in range(NCORES):
        in_maps.append(
            {"xt16": _prep_x(inputs[c * BC : (c + 1) * BC]), "w16": w16, "pows": pows}
        )

    run_kwargs = {}
    if os.environ.get("MEMBRANE_TRACE_DIR"):
        run_kwargs["tmpdir"] = os.environ["MEMBRANE_TRACE_DIR"]
    res = bass_utils.run_bass_kernel_spmd(
        nc, in_maps, core_ids=list(range(NCORES)), **run_kwargs
    )
    LAST_RESULT = res

    syn_full = np.empty((B, T, D), dtype=np.float32)
    mem_full = np.empty((B, T, D), dtype=np.float32)
    for c in range(NCORES):
        r = res.results[c]
        cs = slice(c * BC, (c + 1) * BC)
        syn_full[cs] = _unshard(r["syn"].astype(np.float32))
        vt = _unshard(r["mem"].astype(np.float32))
        mem_full[cs, 1:, :] = vt[:, : T - 1, :] * omb
    syn_full[:, 0, :] = 0.0
    mem_full[:, 0, :] = 0.0
    return (syn_full, mem_full)


# revision 14
# speedup vs baseline: 1.3418x; 1.1292x over previous
"""Trainium2 Bass kernel for nn_MembraneLayer: h = x @ w followed by a
double first-order recurrence over time, producing (syn_rec, mem_rec).

Sharding: data-parallel over batch. 8 cores x 64 batches each.

Per-core layout (hardcoded), columns = b*100 + t per quartet of 16
batches (4 batches per PSUM bank):
  xt16  [700, 6400]   fp16  x transposed to [C, b*T+t] (host-prepped)
  w16   [700, 512]    fp16
  acoef [4, 128, 400] f32   alpha per (d_tile, partition), 0 at t=0 cols
  bcoef [4, 128,1600] f32   beta likewise
  syn   [512, 6400]   fp16  out: syn_rec in [D, b*T+t] layout
  mem   [512, 6400]   fp16  out: v = mem/(1-beta), UNSHIFTED; host applies
                            the (1-beta) scale and the one-step shift

Design notes (measured on HW):
 - single fp16 matmul pass (PE streams ~0.42 ns/col; 6 k-tiles x 4 banks
   per (quartet, d_tile), shifted write to slot t+1, t=0 memset)
 - the DVE serial scan costs ~2.2 ns/col and supports NO 2x perf modes
   (InstTensorScalarPtr reports none); a Blelloch block-scan pyramid of
   full-rate STT ops is NOT cheaper because STT also has no 2x mode and
   per-op overhead (~130 ns) eats the difference: plain scans with
   minimal op count win. GPSIMD cannot run TensorScalarPtr at all.
 - fp16 outputs halve store traffic; scan state stays fp32 internally
   (out is downcast on write), so precision loss is only on stored values
 - scan coefficients come from DRAM (DMA has slack; DVE is the
   bottleneck, so no on-device coef generation)
"""

import os
from contextlib import ExitStack

import numpy as np

import concourse.bass as bass
import concourse.tile as tile
from concourse import bacc, mybir
from concourse import bass_utils

B, T, C, D = 512, 100, 700, 512
NCORES = 8
BC = B // NCORES  # 64 batches per core
NQ = 4  # quartets: 16 batches = 1600 columns each
QCOLS = 1600
KT = [(k * 128, min(128, C - k * 128)) for k in range(6)]
F32 = mybir.dt.float32
FP16 = mybir.dt.float16
MULT = mybir.AluOpType.mult
ADD = mybir.AluOpType.add

MODE = "fp16-plainscan"
LAST_RESULT = None
_cache = {}


def _build(sim_safe=False):
    """sim_safe=True splits each matmul per batch (rank-2 out views) so
    CoreSim's 2-D result assert passes; numerics identical."""
    key = ("nc", sim_safe)
    if key in _cache:
        return _cache[key]
    nc = bacc.Bacc("TRN2", target_bir_lowering=False, debug=False)

    xt_d = nc.dram_tensor("xt16", [C, BC * T], FP16, kind="ExternalInput").ap()
    w_d = nc.dram_tensor("w16", [C, D], FP16, kind="ExternalInput").ap()
    ac_d = nc.dram_tensor("acoef", [4, 128, 400], F32, kind="ExternalInput").ap()
    bc_d = nc.dram_tensor("bcoef", [4, 128, QCOLS], F32, kind="ExternalInput").ap()
    syn_d = nc.dram_tensor("syn", [D, BC * T], FP16, kind="ExternalOutput").ap()
    mem_d = nc.dram_tensor("mem", [D, BC * T], FP16, kind="ExternalOutput").ap()

    with tile.TileContext(nc) as tc:
        with ExitStack() as ctx:
            cpool = ctx.enter_context(tc.tile_pool(name="consts", bufs=1))
            # weights + coefs on the gpsimd (SWDGE) queue so the Sync queue
            # leads with the first x tiles
            w_tiles = []
            for k, (r0_, rk) in enumerate(KT):
                wt = cpool.tile([128, D], FP16, name=f"w{k}", tag=f"w{k}")
                nc.gpsimd.dma_start(wt[:rk, :], w_d[r0_ : r0_ + rk, :])
                w_tiles.append(wt)
            ac_t, bc_t = [], []
            for di in range(4):
                a = cpool.tile([128, 400], F32, name=f"ac{di}", tag=f"ac{di}")
                nc.gpsimd.dma_start(a[:], ac_d[di])
                ac_t.append(a)
                b_ = cpool.tile([128, QCOLS], F32, name=f"bc{di}", tag=f"bc{di}")
                nc.gpsimd.dma_start(b_[:], bc_d[di])
                bc_t.append(b_)

            xp = ctx.enter_context(tc.tile_pool(name="xp", bufs=2))
            pp = ctx.enter_context(tc.tile_pool(name="pp", bufs=2, space="PSUM"))
            sp = ctx.enter_context(tc.tile_pool(name="sp", bufs=2))
            vp = ctx.enter_context(tc.tile_pool(name="vp", bufs=2))

            # PE warmup: dummy matmuls run during the initial DMA wait so HAM
            # un-throttles before the first real MM
            warm_sb = cpool.tile([128, 512], FP16, name="warm", tag="warm")
            nc.vector.memset(warm_sb[:], 0.0)
            warm_ps = pp.tile([128, 2048], F32, tag="ps", name="warm_ps")
            for _ in range(16):
                nc.tensor.matmul(
                    warm_ps[:, 0:512], warm_sb[:, 0:128], warm_sb[:], start=True, stop=True
                )

            for q in range(NQ):
                qc0 = q * QCOLS
                xts = []
                for k, (r0_, rk) in enumerate(KT):
                    t_ = xp.tile([128, QCOLS], FP16, tag=f"x{k}", name=f"x{k}_{q}")
                    nc.sync.dma_start(t_[:rk, :], xt_d[r0_ : r0_ + rk, qc0 : qc0 + QCOLS])
                    xts.append(t_)

                for di in range(4):
                    dsl = slice(di * 128, (di + 1) * 128)

                    # h matmul: 4 batches per PSUM bank, shifted write to t+1
                    ps = pp.tile([128, 2048], F32, tag="ps", name=f"ps_{q}_{di}")
                    for k, (r0_, rk) in enumerate(KT):
                        lhsT = w_tiles[k][:rk, dsl]
                        for g in range(4):
                            if sim_safe:
                                for b_ in range(4):
                                    c0 = g * 400 + b_ * 100
                                    nc.tensor.matmul(
                                        ps[:, g * 512 + b_ * 100 + 1 : g * 512 + b_ * 100 + 100],
                                        lhsT,
                                        xts[k][:rk, c0 : c0 + 99],
                                        start=(k == 0 and b_ == 0),
                                        stop=(k == 5 and b_ == 3),
                                    )
                                continue
                            rhs3 = xts[k][:rk, g * 400 : (g + 1) * 400].rearrange(
                                "p (b t) -> p b t", t=100
                            )[:, :, 0:99]
                            out3 = ps[:, g * 512 : g * 512 + 400].rearrange(
                                "p (b t) -> p b t", t=100
                            )[:, :, 1:100]
                            nc.tensor.matmul(
                                out3, lhsT, rhs3, start=(k == 0), stop=(k == 5)
                            )
                    # h[t=0] := 0 (one rank-3 memset over the 16 t=0 slots)
                    nc.vector.memset(
                        ps.rearrange("p (g x) -> p g x", x=512)[:, :, 0:301:100], 0.0
                    )

                    syn16 = sp.tile([128, QCOLS], FP16, tag="syn", name=f"sy_{q}_{di}")
                    v16 = vp.tile([128, QCOLS], FP16, tag="v", name=f"v_{q}_{di}")
                    for g in range(4):
                        nc.vector.tensor_tensor_scan(
                            syn16[:, g * 400 : (g + 1) * 400],
                            ac_t[di][:],
                            ps[:, g * 512 : g * 512 + 400],
                            0.0,
                            MULT,
                            ADD,
                        )
                    nc.scalar.dma_start(syn_d[dsl, qc0 : qc0 + QCOLS], syn16[:])
                    nc.vector.tensor_tensor_scan(
                        v16[:], bc_t[di][:], syn16[:], 0.0, MULT, ADD
                    )
                    nc.scalar.dma_start(mem_d[dsl, qc0 : qc0 + QCOLS], v16[:])

    nc.compile()
    _cache[key] = nc
    return nc


def kernel(inputs, w, alpha, beta):
    global LAST_RESULT
    inputs = np.asarray(inputs, dtype=np.float32)
    w = np.asarray(w, dtype=np.float32)
    alpha = np.asarray(alpha, dtype=np.float32).reshape(-1)
    beta = np.asarray(beta, dtype=np.float32).reshape(-1)

    nc = _build()

    acoef = np.broadcast_to(
        alpha.reshape(4, 128, 1), (4, 128, 400)
    ).astype(np.float32).copy()
    acoef[:, :, 0::100] = 0.0
    bcoef = np.broadcast_to(
        beta.reshape(4, 128, 1), (4, 128, QCOLS)
    ).astype(np.float32).copy()
    bcoef[:, :, 0::100] = 0.0
    w16 = w.astype(np.float16)
    omb = (1.0 - beta).reshape(1, 1, D)

    in_maps = []
    for c in range(NCORES):
        xc = inputs[c * BC : (c + 1) * BC]  # [64, 100, 700]
        xt16 = xc.reshape(BC * T, C).T.astype(np.float16)  # [700, 6400]
        in_maps.append({"xt16": xt16, "w16": w16, "acoef": acoef, "bcoef": bcoef})

    run_kwargs = {}
    if os.environ.get("MEMBRANE_TRACE_DIR"):
        run_kwargs["tmpdir"] = os.environ["MEMBRANE_TRACE_DIR"]
    res = bass_utils.run_bass_kernel_spmd(
        nc, in_maps, core_ids=list(range(NCORES)), **run_kwargs
    )
    LAST_RESULT = res

    syn_full = np.empty((B, T, D), dtype=np.float32)
    mem_full = np.empty((B, T, D), dtype=np.float32)
    for c in range(NCORES):
        r = res.results[c]
        cs = slice(c * BC, (c + 1) * BC)
        syn_full[cs] = (
            r["syn"].astype(np.float32).reshape(D, BC, T).transpose(1, 2, 0)
        )
        vt = r["mem"].astype(np.float32).reshape(D, BC, T).transpose(1, 2, 0)
        mem_full[cs, 1:, :] = vt[:, : T - 1, :] * omb
    syn_full[:, 0, :] = 0.0
    mem_full[:, 0, :] = 0.0
    return (syn_full, mem_full)
